# revision 68
# baseline (speedup 1.0000x reference)
"""Trainium2 Bass kernel for DomainInvariantFeaturesLearningNetwork.

Computation (reference):
  di  = relu(BN(relu(BN(features @ W1)) @ W2))            # [N, H] node feats
  hi  = di @ We1[:H];  hj = di @ We1[H:]                  # edge-net split GEMMs
  logits[i,j] = relu(hi[i] + hj[j] + bwe1) . we2 + bwe2   # all-pairs edge MLP
  w = where(same_label & offdiag, sigmoid(logits), 0)
  out = di + where(wsum>0, (w @ di) / wsum, 0)

Structure: the same_label mask makes the [N, N] edge matrix block-diagonal
after grouping nodes by label.  Host assigns nodes to 80-padded label
groups (16 groups, max count 75); each core owns 2 groups = 160 query
slots x 80 keys.  The MLP runs replicated per core in transposed [H, N]
space (bf16 feature path; pre-BN biases cancel under BN and are dropped).

Edge stage (instruction-count-optimized): per (slot s, h-chunk hc) one
fused DVE/Pool/Act tensor_scalar produces relu(hjT + hi_s + bwe1) as a
bf16 [128h, 80k] chunk of a [128h, 400] five-slot moving tile, with one
producer engine per tile (single semaphore for the consumer).  One PE
matmul per (row r, hc) contracts that tile with a host-built stationary
that has we2[hc] in column r%16 and zeros elsewhere: the matmul
accumulates row r of a per-group [16, 400] logits PSUM tile and adds
exact zeros to every other row.  64 wide matmuls replace the original
512 Ldweights + 512 single-column matmuls.

The per-group PSUM split lets group 0's whole epilogue (sigmoid + diag/
pad mask, 5 PE transposes into one PSUM bank, single copy out, one
[w^T | keys]x[di | ones] matmul producing both w @ di and the row sums,
normalize + residual add, output DMA) overlap group 1's edge matmuls.
Epilogue rows come out in transpose-native (j, r) order; the host maps
them back to nodes (pi_slot).  di enters the final add via a second
one-hot gather in that same order (keysel2).
"""

import numpy as np
import ml_dtypes

import concourse.bass as bass
import concourse.tile as tile
from concourse import mybir
from concourse.bass_utils import run_bass_kernel_spmd

FP32 = mybir.dt.float32
F32R = mybir.dt.float32r
BF16 = mybir.dt.bfloat16
AF = mybir.ActivationFunctionType
OP = mybir.AluOpType

N = 1024          # nodes
FD = 2048         # feature dim
H = 256           # hidden dim (2 partition chunks)
NCORES = 8
P = 128
NG = 16           # label groups
GPAD = 80         # padded group size (slots per group; max count is 75)
GPC = NG // NCORES  # groups per core (2)
QS = GPC * GPAD   # query slots per core (160)
SPR = 5           # slots per PSUM row (5 x 80 = 400 free; 80/5=16
                  # rows per group, so group rows start at partition 0)
NROW = QS // SPR  # 32 logits rows
FREE = SPR * GPAD  # 400
BN_EPS = 1e-5
PAIR_BUFS = 16
NWARM = 0        # PE p-state warmup matmuls during the DMA-load window

_CACHE = {}


def _patch_drain():
    """walrus in this container rejects >1 sync wait on a CTRL instruction;
    split the tile-exit drain waits across sync NOPs, one wait each."""
    if getattr(tile.TileContext, "_drain_patched", False):
        return
    from concourse.tile import ScopedClock

    def _patched(self, tick_clock, wait_clock):
        nop0 = self.nc.sync.nop(nofuse=True, hint="pre_drain_waits")
        wait_clock.add_sem_waits(
            nop0.ins, ScopedClock({None: tick_clock.global_clock})
        )
        si = nop0.ins.sync_info
        if si and si.on_wait and len(si.on_wait) > 1:
            waits = list(si.on_wait)
            si.on_wait = waits[:1]
            for i in range(1, len(waits)):
                nk = self.nc.sync.nop(nofuse=True, hint=f"pre_drain_w{i}")
                nsi = nk.ins.sync_info
                if nsi is None:
                    nk.ins.sync_info = mybir.SyncInfo(
                        on_wait=waits[i : i + 1], on_update=[]
                    )
                else:
                    nsi.on_wait = waits[i : i + 1]
        self.nc.sync.drain()
        self.nc.all_engine_barrier()
        assert self.sems is not None
        popped = self.nc._tile_sem_poison_stack.pop()
        assert popped is self._sem_poison
        self.nc.clear_and_free_semaphores(list(self.sems.allocated().values()))
        self.nc.all_engine_barrier()

    tile.TileContext._drain_and_barrier = _patched
    tile.TileContext._drain_patched = True


def _split_multi_waits(nc):
    """walrus here accepts at most one sync-wait per instruction; hoist
    extras onto same-engine NOPs inserted immediately before (and before
    any contiguous LDWEIGHTS run, so the weight load can't slip past)."""
    idx = 0
    for bb in nc.main_func.blocks:
        new_insts = []
        changed = False
        for ins in bb.instructions:
            si = ins.sync_info
            if si is not None and si.on_wait and len(si.on_wait) > 1:
                waits = list(si.on_wait)
                ip = len(new_insts)
                while (
                    ip > 0
                    and isinstance(new_insts[ip - 1], mybir.InstLdweights)
                    and new_insts[ip - 1].engine == ins.engine
                ):
                    ip -= 1
                for w in waits[:-1]:
                    idx += 1
                    nop = mybir.InstNoOp(
                        name=f"waitsplit_{idx}",
                        engine=ins.engine,
                        sync_info=mybir.SyncInfo(on_wait=[w], on_update=[]),
                        bass_nofuse=True,
                    )
                    nc.register_instruction(nop)
                    new_insts.insert(ip, nop)
                    ip += 1
                si.on_wait = waits[-1:]
                changed = True
            new_insts.append(ins)
        if changed:
            bb.instructions = new_insts


def _bn_scales(nc, small, st, g_col, bt_col, eps_t, ht):
    """Training-mode BN scale/shift columns from precomputed bn_stats
    pairs (one per 512-column half).  Returns (scale, shift) [128,1]."""
    mv = small.tile([P, 2], FP32, tag="bn_mv")
    nc.vector.bn_aggr(mv, st)
    sd = small.tile([P, 1], FP32, tag="bn_sd")
    nc.scalar.activation(sd, mv[:, 1:2], AF.Sqrt, bias=eps_t[:])
    rinv = small.tile([P, 1], FP32, tag="bn_rinv")
    nc.vector.reciprocal(rinv, sd)
    scale = small.tile([P, 1], FP32, tag="bn_scale")
    nc.vector.tensor_mul(scale, rinv, g_col[:, ht : ht + 1])
    ms = small.tile([P, 1], FP32, tag="bn_ms")
    nc.vector.tensor_mul(ms, mv[:, 0:1], scale)
    shift = small.tile([P, 1], FP32, tag="bn_shift")
    nc.vector.tensor_sub(shift, bt_col[:, ht : ht + 1], ms)
    return scale, shift


def _build_program(reps=1):
    _patch_drain()
    nc = bass.Bass()

    featT = nc.declare_dram_parameter("featT", [FD, N], BF16, isOutput=False)
    W1 = nc.declare_dram_parameter("W1", [FD, H], BF16, isOutput=False)
    W2 = nc.declare_dram_parameter("W2", [H, H], BF16, isOutput=False)
    We1a = nc.declare_dram_parameter("We1a", [H, H], BF16, isOutput=False)
    We1b = nc.declare_dram_parameter("We1b", [H, H], BF16, isOutput=False)
    bwe1 = nc.declare_dram_parameter("bwe1", [H], FP32, isOutput=False)
    bwe2 = nc.declare_dram_parameter("bwe2", [1], FP32, isOutput=False)
    g1 = nc.declare_dram_parameter("g1", [H], FP32, isOutput=False)
    bt1 = nc.declare_dram_parameter("bt1", [H], FP32, isOutput=False)
    g2 = nc.declare_dram_parameter("g2", [H], FP32, isOutput=False)
    bt2 = nc.declare_dram_parameter("bt2", [H], FP32, isOutput=False)
    keysel = nc.declare_dram_parameter("keysel", [N, QS], BF16, isOutput=False)
    keysel2 = nc.declare_dram_parameter("keysel2", [N, QS], BF16,
                                        isOutput=False)
    statw = nc.declare_dram_parameter(
        "statw", [P, 2, NROW, GPAD // SPR], BF16, isOutput=False
    )
    maskq = nc.declare_dram_parameter("maskq", [GPAD, GPC, GPAD], BF16,
                                      isOutput=False)
    ident = nc.declare_dram_parameter("ident", [P, P], BF16, isOutput=False)
    out_block = nc.declare_dram_parameter(
        "out_block", [QS, H], FP32, isOutput=True
    )

    from contextlib import ExitStack

    with tile.TileContext(nc) as tc, ExitStack() as ctx:
        const = ctx.enter_context(tc.tile_pool(name="const", bufs=1))
        persist = ctx.enter_context(tc.tile_pool(name="persist", bufs=1))
        small = ctx.enter_context(tc.tile_pool(name="small", bufs=2))
        feat_pool = ctx.enter_context(tc.tile_pool(name="feat", bufs=6))

        # ---- weight + feature loads (sync queue: critical path) --------
        # interleave W1 chunk loads with feature chunk loads so the first
        # GEMM matmuls can start after ~2 small transfers
        W1r = const.tile([P, FD // P, H], BF16)
        W1v = W1[:].rearrange("(c p) h -> p c h", p=P)
        ftrs = [const.tile([P, N], BF16, tag=f"ftr{k}", name=f"ftr{k}")
                for k in range(FD // P)]
        for k in range(FD // P):
            if k % 4 == 0:
                nc.sync.dma_start(out=W1r[:, k : k + 4, :],
                                  in_=W1v[:, k : k + 4, :])
            nc.sync.dma_start(out=ftrs[k][:], in_=featT[k * P : (k + 1) * P, :])
        # remaining constants: allocate now, DMA after the GEMM1 matmuls are
        # queued so these transfers don't steal early DMA bandwidth from the
        # feature stream (gpsimd queue, which is otherwise idle early)
        W2r = const.tile([P, H // P, H], BF16)
        We1ar = const.tile([P, H // P, H], BF16)
        We1br = const.tile([P, H // P, H], BF16)
        cols = {n: const.tile([P, 2], FP32, tag=f"col_{n}", name=f"c_{n}")
                for n in ("g1", "bt1", "g2", "bt2", "bwe1")}
        bwe2_col = const.tile([GPAD // SPR, 1], FP32)
        eps_t = const.tile([P, 1], FP32)
        nc.vector.memset(eps_t[:], BN_EPS)
        # PE p-state warmup: dummy matmuls on memset scratch keep the PE
        # continuously busy through the initial DMA window so GEMM1 starts
        # at full clock (2.4GHz) instead of ramping from 0.65GHz
        if NWARM:
            warm = const.tile([P, 640], BF16)
            nc.vector.memset(warm[:], 0.0)
            with tc.tile_pool(name="warm_ps", bufs=1, space="PSUM") as wps:
                wp = wps.tile([P, 512], FP32)
                for _ in range(NWARM):
                    nc.tensor.matmul(
                        wp[:], warm[:, 0:P], warm[:, P : P + 512],
                        start=True, stop=True,
                    )
        ident_b = const.tile([P, P], BF16)
        keysel_b = const.tile([P, N // P, QS], BF16)
        keysel2_b = const.tile([P, N // P, QS], BF16)
        stat_sb = const.tile([P, 2, NROW, GPAD // SPR], BF16)
        mask_sb = const.tile([GPAD, GPC, GPAD], BF16)

        def _emit_const_dmas():
            # on the SP hardware queue so they line up BEHIND the feature
            # stream (the gpsimd/Pool software-DGE queue starts at t=0 and
            # would steal DMA-engine bandwidth from the critical loads)
            nc.sync.dma_start(
                out=W2r[:], in_=W2[:].rearrange("(c p) h -> p c h", p=P)
            )
            nc.sync.dma_start(
                out=We1ar[:], in_=We1a[:].rearrange("(c p) h -> p c h", p=P)
            )
            nc.sync.dma_start(
                out=We1br[:], in_=We1b[:].rearrange("(c p) h -> p c h", p=P)
            )
            for n, v in (("g1", g1), ("bt1", bt1), ("g2", g2), ("bt2", bt2),
                         ("bwe1", bwe1)):
                nc.sync.dma_start(out=cols[n][:],
                                  in_=v[:].rearrange("(c p) -> p c", p=P))
            nc.sync.dma_start(
                out=bwe2_col[:],
                in_=bass.AP(tensor=bwe2[:].tensor, offset=0,
                            ap=[[0, GPAD // SPR], [1, 1]]),
            )
            nc.sync.dma_start(out=ident_b[:], in_=ident[:])
            nc.sync.dma_start(
                out=keysel_b[:],
                in_=keysel[:].rearrange("(c p) s -> p c s", p=P),
            )
            nc.sync.dma_start(
                out=keysel2_b[:],
                in_=keysel2[:].rearrange("(c p) s -> p c s", p=P),
            )
            nc.sync.dma_start(out=stat_sb[:], in_=statw[:])
            nc.sync.dma_start(out=mask_sb[:], in_=maskq[:])

        for rep in range(reps):
            # ---- MLP in transposed space -------------------------------
            # ht-outer loops so BN of chunk 0 overlaps the GEMM of chunk 1
            h1T = [persist.tile([P, N], BF16, tag=f"h1T{t}", name=f"h1T{t}")
                   for t in range(2)]
            diT = [persist.tile([P, N], BF16, tag=f"diT{t}", name=f"diT{t}")
                   for t in range(2)]

            di_nat = persist.tile([P, N // P, H], BF16, tag="di_nat")
            diT_keys = persist.tile([P, 2, QS], BF16, tag="diT_keys")
            # moving tiles per group: [di_keys | ones] bf16
            mg = [persist.tile([GPAD, H + 1], BF16, tag=f"mg{g}",
                               name=f"mg{g}") for g in range(GPC)]

            def _copy(i, out, in_):
                # gpsimd (Pool) cannot access PSUM; split DVE/Act 3:1
                if i % 4 == 3:
                    nc.scalar.copy(out, in_)
                else:
                    nc.vector.tensor_copy(out, in_)

            with tc.tile_pool(name=f"mlp_ps_r{rep}", bufs=2,
                              space="PSUM") as mlp_ps, \
                 tc.tile_pool(name=f"tr_ps_r{rep}", bufs=2,
                              space="PSUM") as tr_ps:
                psum_x = [mlp_ps.tile([P, N], FP32, tag="big",
                                      name=f"psum_x{t}") for t in range(2)]
                # k-outer so PE consumes each feature chunk with 4 matmuls
                # (stays ahead of the DMA stream)
                for k in range(FD // P):
                    for ht in range(2):
                        for nh in range(2):
                            nc.tensor.matmul(
                                psum_x[ht][:, nh * 512 : (nh + 1) * 512],
                                W1r[:, k, ht * P : (ht + 1) * P],
                                ftrs[k][:, nh * 512 : (nh + 1) * 512],
                                start=(k == 0),
                                stop=(k == FD // P - 1),
                            )
                if rep == 0:
                    _emit_const_dmas()
                # BN1: stats, scale/shift, then node-half applies so the
                # GEMM2 matmuls on half 0 overlap the half-1 applies
                st_x = []
                for ht in range(2):
                    st = small.tile([P, 2, 6], FP32, tag="bn_st",
                                    name=f"stx{ht}")
                    for nh in range(2):
                        nc.vector.bn_stats(
                            st[:, nh, :],
                            psum_x[ht][:, nh * 512 : (nh + 1) * 512],
                        )
                    st_x.append(st)
                # scale chains after ALL stats: the chain's Act round-trips
                # would otherwise stall ht1's stats behind ht0 on DVE
                ssx = [_bn_scales(nc, small, st_x[ht], cols["g1"],
                                  cols["bt1"], eps_t, ht) for ht in range(2)]
                # BN1 scale-fold: relu(x*s + b) = s*relu(x + b/s) for s > 0
                # (gamma1 is ones), with s folded into the W2 stationary
                # rows -- the applies then split across Act AND DVE instead
                # of serializing on Act
                shp = []
                for ht in range(2):
                    scale, shift = ssx[ht]
                    sre = small.tile([P, 1], FP32, tag="bn_sre",
                                     name=f"sre{ht}")
                    nc.vector.reciprocal(sre[:], scale[:])
                    sp = small.tile([P, 1], FP32, tag="bn_shp",
                                    name=f"shp{ht}")
                    nc.vector.tensor_mul(sp[:], shift[:], sre[:])
                    shp.append(sp)
                W2s = persist.tile([P, H // P, H], BF16, tag="W2s")
                for k in range(2):
                    nc.gpsimd.tensor_scalar(
                        out=W2s[:, k, :], in0=W2r[:, k, :],
                        scalar1=ssx[k][0][:], scalar2=None, op0=OP.mult,
                    )
                psum_y = [mlp_ps.tile([P, N], FP32, tag="big",
                                      name=f"psum_y{t}") for t in range(2)]
                st_y = [small.tile([P, 2, 6], FP32, tag="bn_st",
                                   name=f"sty{t}") for t in range(2)]
                for nh in range(2):
                    sl = slice(nh * 512, (nh + 1) * 512)
                    for ht in range(2):
                        if ht == 0:
                            nc.scalar.activation(
                                h1T[ht][:, sl], psum_x[ht][:, sl], AF.Relu,
                                bias=shp[ht][:],
                            )
                        else:
                            nc.vector.tensor_scalar(
                                out=h1T[ht][:, sl], in0=psum_x[ht][:, sl],
                                scalar1=shp[ht][:], scalar2=0.0,
                                op0=OP.add, op1=OP.max,
                            )
                    # k-outer: the k=0 matmuls only need h1T[0] (apply of
                    # ht=0), so they run while the ht=1 apply is still going
                    for k in range(2):
                        for ht in range(2):
                            nc.tensor.matmul(
                                psum_y[ht][:, sl],
                                W2s[:, k, ht * P : (ht + 1) * P],
                                h1T[k][:, sl],
                                start=(k == 0),
                                stop=(k == 1),
                            )
                    for ht in range(2):
                        nc.vector.bn_stats(st_y[ht][:, nh, :],
                                           psum_y[ht][:, sl])

                # BN2 scale/shift, node-half applies + per-half transposes;
                # ht=1 applies run on DVE (mult+add then relu) in parallel
                # with ht=0 on Act
                bn2tmp = persist.tile([P, 512], BF16, tag="bn2tmp")
                for ht in range(2):
                    scale, shift = _bn_scales(nc, small, st_y[ht], cols["g2"],
                                              cols["bt2"], eps_t, ht)
                    for nh in range(2):
                        sl = slice(nh * 512, (nh + 1) * 512)
                        if ht == 0:
                            nc.scalar.activation(
                                diT[ht][:, sl], psum_y[ht][:, sl], AF.Relu,
                                bias=shift[:], scale=scale[:],
                            )
                        else:
                            nc.vector.tensor_scalar(
                                out=bn2tmp[:], in0=psum_y[ht][:, sl],
                                scalar1=scale[:], scalar2=shift[:],
                                op0=OP.mult, op1=OP.add,
                            )
                            nc.vector.tensor_scalar(
                                out=diT[ht][:, sl], in0=bn2tmp[:],
                                scalar1=0.0, scalar2=None, op0=OP.max,
                            )
                        # spread the 4 transposes across distinct HWDGE
                        # queues -- on one queue the 632ns configs serialize
                        teng = (nc.scalar, nc.sync)[nh]
                        teng.dma_start_transpose(
                            out=di_nat[:, nh * 4 : (nh + 1) * 4,
                                       ht * P : (ht + 1) * P],
                            in_=diT[ht][:, sl],
                        )
                    # diT_keys[h, slot] one-hot gather for this h-chunk
                    pdk = tr_ps.tile([P, QS], FP32, tag="sm", name=f"pdk{ht}")
                    for jb in range(N // P):
                        nc.tensor.matmul(
                            pdk[:],
                            di_nat[:, jb, ht * P : (ht + 1) * P],
                            keysel_b[:, jb, :],
                            start=(jb == 0),
                            stop=(jb == N // P - 1),
                        )
                    nc.vector.tensor_copy(diT_keys[:, ht, :], pdk[:])

            # ---- hj / hi (critical path to edge), then key blocks ------
            with tc.tile_pool(name=f"gat_ps_r{rep}", bufs=2,
                              space="PSUM") as gat_ps:
                hjT_keys = persist.tile([P, 2, QS], BF16, tag="hjT_keys")
                bias_all = persist.tile([P, 2, QS], FP32, tag="bias_all")
                for ht in range(2):
                    phj = gat_ps.tile([P, QS], FP32, tag="sm", name=f"phj{ht}")
                    for k in range(2):
                        nc.tensor.matmul(
                            phj[:],
                            We1br[:, k, ht * P : (ht + 1) * P],
                            diT_keys[:, k, :],
                            start=(k == 0),
                            stop=(k == 1),
                        )
                    nc.scalar.copy(hjT_keys[:, ht, :], phj[:])
                    phi = gat_ps.tile([P, QS], FP32, tag="sm", name=f"phi{ht}")
                    for k in range(2):
                        nc.tensor.matmul(
                            phi[:],
                            We1ar[:, k, ht * P : (ht + 1) * P],
                            diT_keys[:, k, :],
                            start=(k == 0),
                            stop=(k == 1),
                        )
                    nc.vector.tensor_scalar(
                        out=bias_all[:, ht, :], in0=phi[:],
                        scalar1=cols["bwe1"][:, ht : ht + 1], scalar2=None,
                        op0=OP.add,
                    )

                di_pi = [persist.tile([GPAD, H], BF16, tag=f"di_pi{g}",
                                      name=f"di_pi{g}") for g in range(GPC)]
                for g in range(GPC):
                    pb = gat_ps.tile([GPAD, H], FP32, tag="kb", name=f"kb{g}")
                    for jb in range(N // P):
                        nc.tensor.matmul(
                            pb[:],
                            keysel_b[:, jb, g * GPAD : (g + 1) * GPAD],
                            di_nat[:, jb, :],
                            start=(jb == 0),
                            stop=(jb == N // P - 1),
                        )
                    _copy(g, mg[g][:, 0:H], pb[:])
                    nc.gpsimd.memset(mg[g][:, H : H + 1], 1.0)
                    # same gather in pi (transpose-output) slot order, used
                    # by the final add so it aligns with the epilogue rows
                    pq = gat_ps.tile([GPAD, H], FP32, tag="kb", name=f"kq{g}")
                    for jb in range(N // P):
                        nc.tensor.matmul(
                            pq[:],
                            keysel2_b[:, jb, g * GPAD : (g + 1) * GPAD],
                            di_nat[:, jb, :],
                            start=(jb == 0),
                            stop=(jb == N // P - 1),
                        )
                    _copy(g + 1, di_pi[g][:], pq[:])

            # ---- edge stage + per-group epilogue -----------------------
            # separate PSUM accumulation per group: group 0's epilogue
            # (sigmoid/mask/transpose/aggregate/out) overlaps group 1's
            # edge matmuls
            RPG = GPAD // SPR  # 16 logits rows per group
            with (
                tc.tile_pool(name=f"edge_ps_r{rep}", bufs=2,
                             space="PSUM") as edge_ps,
                tc.tile_pool(name=f"ep_ps_r{rep}", bufs=2,
                             space="PSUM") as ep_ps,
                tc.tile_pool(name=f"pair_pool_r{rep}",
                             bufs=PAIR_BUFS) as pair_pool,
            ):
                # one producer engine per pair tile (1 sem for the matmul);
                # greedy assignment by measured per-op engine cost
                eng_cost = [(nc.vector, 97.0), (nc.gpsimd, 230.0),
                            (nc.scalar, 290.0)]
                eng_load = [0.0, 0.0, 0.0]
                ui = 0
                for g in range(GPC):
                    logits_ps = edge_ps.tile([RPG, FREE], FP32, tag="logits",
                                             name=f"logits{g}")
                    nmm = RPG * 2
                    mi = 0
                    for r in range(RPG):
                        for hc in range(2):
                            ui += 1
                            ei = min(range(3),
                                     key=lambda x: eng_load[x] + eng_cost[x][1])
                            eng_load[ei] += SPR * eng_cost[ei][1]
                            eng = eng_cost[ei][0]
                            pair = pair_pool.tile([P, FREE], BF16, tag="pair",
                                                  name=f"pair{g}_{r}_{hc}")
                            for j in range(SPR):
                                s = g * GPAD + r * SPR + j
                                if eng is nc.scalar:
                                    nc.scalar.activation(
                                        out=pair[:, j * GPAD : (j + 1) * GPAD],
                                        in_=hjT_keys[
                                            :, hc, g * GPAD : (g + 1) * GPAD
                                        ],
                                        func=AF.Relu,
                                        bias=bias_all[:, hc, s : s + 1],
                                    )
                                else:
                                    eng.tensor_scalar(
                                        out=pair[:, j * GPAD : (j + 1) * GPAD],
                                        in0=hjT_keys[
                                            :, hc, g * GPAD : (g + 1) * GPAD
                                        ],
                                        scalar1=bias_all[:, hc, s : s + 1],
                                        scalar2=0.0,
                                        op0=OP.add, op1=OP.max,
                                    )
                            nc.tensor.matmul(
                                logits_ps[:],
                                stat_sb[:, hc, g * RPG + r, :],
                                pair[:],
                                start=(mi == 0),
                                stop=(mi == nmm - 1),
                            )
                            mi += 1

                    # -- epilogue for this group (overlaps next group) --
                    wfin = persist.tile([RPG, FREE], BF16, tag=f"wfin{g}",
                                        name=f"wfin{g}")
                    nc.scalar.activation(
                        wfin[:], logits_ps[:], AF.Sigmoid, bias=bwe2_col[:]
                    )

                    # 5 transposes into one PSUM tile, single copy out;
                    # stationary free order is (j, r) -- pi slot order,
                    # matching keysel2/di_pi and unscrambled on the host
                    pst = ep_ps.tile([GPAD, SPR, RPG], BF16, tag="wtr",
                                     name=f"wtr{g}")
                    for j in range(SPR):
                        nc.tensor.transpose(
                            pst[:, j, :],
                            wfin[:, j * GPAD : (j + 1) * GPAD],
                            ident_b[0:RPG, 0:RPG],
                        )
                    # masked copy out of PSUM: mask (diag + pad, transposed
                    # pi layout) folds into the copy, no separate mask op
                    wT = persist.tile([GPAD, SPR * RPG], BF16, tag=f"wT{g}",
                                      name=f"wT{g}")
                    nc.vector.tensor_mul(wT[:], pst[:].rearrange("p a b -> p (a b)"),
                                         mask_sb[:, g, :])

                    pu = ep_ps.tile([GPAD, H + 1], FP32,
                                    tag="upd", name=f"pu{g}")
                    nc.tensor.matmul(
                        pu[:], wT[:], mg[g][:],
                        start=True, stop=True,
                    )
                    wsum = small.tile([GPAD, 1], FP32, tag="wsum",
                                      name=f"ws{g}")
                    nc.vector.tensor_scalar(
                        out=wsum[:], in0=pu[:, H : H + 1],
                        scalar1=1e-30, scalar2=None, op0=OP.max,
                    )
                    rden = small.tile([GPAD, 1], FP32, tag="rden",
                                      name=f"rd{g}")
                    nc.vector.reciprocal(rden[:], wsum[:])
                    tsc = persist.tile([GPAD, H], FP32, tag="tsc",
                                       name=f"tsc{g}")
                    if g == 0:
                        nc.vector.tensor_scalar(
                            out=tsc[:], in0=pu[:, 0:H],
                            scalar1=rden[:], scalar2=None, op0=OP.mult,
                        )
                    else:
                        nc.scalar.activation(
                            out=tsc[:], in_=pu[:, 0:H], func=AF.Copy,
                            scale=rden[:],
                        )
                    out_sb = persist.tile([GPAD, H], FP32, tag="out_sb",
                                          name=f"osb{g}")
                    nc.vector.tensor_add(out_sb[:], tsc[:], di_pi[g][:])
                    nc.sync.dma_start(
                        out=out_block[g * GPAD : (g + 1) * GPAD, :],
                        in_=out_sb[:],
                    )

    _split_multi_waits(nc)
    return nc


def _get_program(reps=1):
    key = f"nc{reps}"
    if key not in _CACHE:
        _CACHE[key] = _build_program(reps)
    return _CACHE[key]


def _host_prep(features, labels, W1, g1, bt1, W2, g2, bt2, We1, bwe1, We2,
               bwe2):
    features = np.asarray(features, dtype=np.float32)
    labels = np.asarray(labels).astype(np.int64)
    We1 = np.asarray(We1, dtype=np.float32)
    we2 = np.asarray(We2, dtype=np.float32)[:, 0]

    # group nodes by label; slot s = GPAD*g + rank within label
    order = np.argsort(labels, kind="stable")
    counts = np.bincount(labels, minlength=NG)
    if counts.max() > GPAD:
        raise ValueError(f"label group too large: {counts.max()} > {GPAD}")
    starts = np.concatenate([[0], np.cumsum(counts)])
    slot2node = np.full(NG * GPAD, -1, dtype=np.int64)
    for v in range(NG):
        cnt = int(counts[v])
        slot2node[v * GPAD : v * GPAD + cnt] = order[starts[v] : starts[v] + cnt]

    bf = ml_dtypes.bfloat16
    RPG = GPAD // SPR
    # stationary bank: statw[p, hc, g*RPG + r, c] = we2[hc*128+p] iff c == r
    statw = np.zeros((P, 2, NROW, RPG), dtype=np.float32)
    for hc in range(2):
        for r in range(NROW):
            statw[:, hc, r, r % RPG] = we2[hc * P : (hc + 1) * P]
    # pi order: epilogue output row 16*j + r  <->  slot 5*r + j (per group)
    rows = np.arange(QS)
    pi_slot = (rows // GPAD) * GPAD + 5 * (rows % GPAD % RPG) + (
        rows % GPAD // RPG
    )

    base = {
        "featT": np.ascontiguousarray(features.T).astype(bf),
        "W1": np.asarray(W1, dtype=np.float32).astype(bf),
        "W2": np.asarray(W2, dtype=np.float32).astype(bf),
        "We1a": We1[:H].astype(bf),
        "We1b": We1[H:].astype(bf),
        "bwe1": np.asarray(bwe1, dtype=np.float32),
        "bwe2": np.asarray(bwe2, dtype=np.float32).reshape(1),
        "g1": np.asarray(g1, dtype=np.float32),
        "bt1": np.asarray(bt1, dtype=np.float32),
        "g2": np.asarray(g2, dtype=np.float32),
        "bt2": np.asarray(bt2, dtype=np.float32),
        "ident": np.eye(P, dtype=np.float32).astype(bf),
        "statw": statw.astype(bf),
    }
    in_maps = []
    for c in range(NCORES):
        lo = c * QS
        slots = slot2node[lo : lo + QS]
        real = slots >= 0
        ksel = np.zeros((N, QS), dtype=np.float32)
        ksel[slots[real], np.nonzero(real)[0]] = 1.0
        # transposed-pi-layout mask: maskT[k, g, col] with col = 16j + r
        # <-> in-group slot 5r + j; 1 iff slot real, key real, key != slot
        mT = np.zeros((GPAD, GPC, GPAD), dtype=np.float32)
        for g in range(GPC):
            for col in range(GPAD):
                i = pi_slot[g * GPAD + col] - g * GPAD
                s = g * GPAD + i
                if not real[s]:
                    continue
                kreal = real[g * GPAD : (g + 1) * GPAD].astype(np.float32)
                kreal = kreal.copy()
                kreal[i] = 0.0
                mT[:, g, col] = kreal
        mm = dict(base)
        mm["keysel"] = ksel.astype(bf)
        mm["keysel2"] = ksel[:, pi_slot].astype(bf)
        mm["maskq"] = mT.astype(bf)
        in_maps.append(mm)
    return in_maps, slot2node, pi_slot


def kernel(features, labels, W1, b1, g1, bt1, W2, b2, g2, bt2,
           We1, bwe1, We2, bwe2, **_unused):
    nc = _get_program()
    in_maps, slot2node, pi_slot = _host_prep(
        features, labels, W1, g1, bt1, W2, g2, bt2, We1, bwe1, We2, bwe2
    )
    _CACHE["last_in_maps"] = in_maps
    res = run_bass_kernel_spmd(nc, in_maps, list(range(NCORES)))
    _CACHE["last_result"] = res
    out = np.empty((N, H), dtype=np.float32)
    for c in range(NCORES):
        blk = res.results[c]["out_block"]
        # device rows are pi-ordered: row i holds slot pi_slot[i]
        slots = slot2node[c * QS : (c + 1) * QS][pi_slot]
        real = slots >= 0
        out[slots[real]] = blk[real]
    return out
